# revision 1
# baseline (speedup 1.0000x reference)
"""Trainium2 Bass kernel for nn_Attention_18949395710608.

Multi-head causal self-attention, B=4, S=2048, D=1024, H=16, dk=dv=64.

Sharding: 8 cores = 4 batches x 2 head-groups (8 heads each).
Each core computes a partial output projection over its 8 heads for its
batch; the host sums the two partials per batch (the "all-reduce").

Per-core kernel (all matmuls bf16 with fp32 PSUM accumulation):
  - Q^T, K^T projections in (head_dim, seq) layout, V in (seq, head_dim)
    layout, all produced from x^T with single big matmuls.  Q/K
    projections for head-pair p+1 are emitted after the attention of
    pair p so the PE has filler work while attention waits on exp.
  - scores^T chunks (128 keys x 512 queries x 2 heads in one 2-bank
    psum tile) per head pair; causal skipping at tile granularity plus
    one (128, 2x128) triangular mask multiply on diagonal blocks.
    exp on ScalarE with the 1/sqrt(dk) scale folded in.
  - AV matmul contracts keys (partition dim) with V carrying an extra
    ones-column, so softmax denominators fall out of row 64 for free.
  - normalization: fast-reciprocal on DVE (custom DVE ops only work at
    partition base 0, so sums are first copied to rows 0/32 of a
    base-0 tile), then a DRAM-bounce DMA broadcasts the per-query
    reciprocals across partitions; DVE multiplies into OT.
  - output projection over the 8 local heads -> fp32 partial (2048, 1024).
"""

import math

import numpy as np
import ml_dtypes

B, S, D, H, DK = 4, 2048, 1024, 16, 64
HL = H // 2          # heads per core
HDL = HL * DK        # 512 local head dims
P = 128
NKT = D // P         # 8 k-tiles over d_in
NPT = HDL // P       # 4 partition tiles over local head dims (head pairs)
NST = S // P         # 16 seq tiles
QC = 512             # query chunk
NQC = S // QC        # 4 query chunks
SCALE = 1.0 / math.sqrt(DK)

BF16 = ml_dtypes.bfloat16

_CACHED = {}


def _build_nc(debug=False):
    import concourse.bass as bass
    import concourse.bacc as bacc
    import concourse.tile as tile
    from concourse import mybir

    bf = mybir.dt.bfloat16
    f32 = mybir.dt.float32

    nc = bacc.Bacc(None, target_bir_lowering=False)

    xT_d = nc.dram_tensor("xT", [D, S], bf, kind="ExternalInput")
    wq_d = nc.dram_tensor("wq", [D, HDL], bf, kind="ExternalInput")
    wk_d = nc.dram_tensor("wk", [D, HDL], bf, kind="ExternalInput")
    wv_d = nc.dram_tensor("wv", [D, HDL], bf, kind="ExternalInput")
    wo_d = nc.dram_tensor("wo", [HDL, D], bf, kind="ExternalInput")
    mask_d = nc.dram_tensor("mask", [P, 2 * P], bf, kind="ExternalInput")
    out_d = nc.dram_tensor("out", [S, D], f32, kind="ExternalOutput")
    if debug:
        dbg_qt_d = nc.dram_tensor("dbg_qt", [P, NPT, S], bf, kind="ExternalOutput")
        dbg_kt_d = nc.dram_tensor("dbg_kt", [P, NPT, S], bf, kind="ExternalOutput")
        dbg_v_d = nc.dram_tensor("dbg_v", [P, NST, HL, 66], bf, kind="ExternalOutput")
        dbg_ot_d = nc.dram_tensor("dbg_ot", [P, NPT, S], bf, kind="ExternalOutput")
        dbg_rec_d = nc.dram_tensor("dbg_rec", [P, QC], f32, kind="ExternalOutput")
        dbg_av_d = nc.dram_tensor("dbg_av", [P, QC], f32, kind="ExternalOutput")
        dbg_pb_d = nc.dram_tensor("dbg_pb", [P, QC], bf, kind="ExternalOutput")

    xT_v = xT_d[:, :].rearrange("(t p) s -> p t s", p=P)
    wq_v = wq_d[:, :].rearrange("(t p) m -> p t m", p=P)
    wk_v = wk_d[:, :].rearrange("(t p) m -> p t m", p=P)
    wv_v = wv_d[:, :].rearrange("(t p) m -> p t m", p=P)
    wo_v = wo_d[:, :].rearrange("(t p) n -> p t n", p=P)
    out_v = out_d[:, :].rearrange("(t p) n -> p t n", p=P)

    with tile.TileContext(nc) as tc:
        with (
            tc.tile_pool(name="consts", bufs=1) as consts,
            tc.tile_pool(name="big", bufs=1) as bigpool,
            tc.tile_pool(name="probs", bufs=8) as ppool,
            tc.tile_pool(name="small", bufs=6) as spool,
            tc.tile_pool(name="osb", bufs=6) as opool,
            tc.tile_pool(name="dramp", bufs=4, space="DRAM") as dramp,
            tc.tile_pool(name="ps_sc", bufs=2, space="PSUM") as ps_sc,
            tc.tile_pool(name="ps_av", bufs=4, space="PSUM") as ps_av,
            tc.tile_pool(name="recp", bufs=4) as recp,
        ):
            # ---- constant / persistent tiles + input DMAs ----
            # per-kt tiles give fine-grained DMA->matmul dependencies;
            # wv/wo loads are emitted later (wv needed ~30us in, wo at the
            # end) so the critical xT/wq/wk transfers get the queues first.
            xt_t = [consts.tile([P, S], bf, name=f"xt{kt}") for kt in range(NKT)]
            wq_t = [consts.tile([P, HDL], bf, name=f"wqt{kt}") for kt in range(NKT)]
            wk_t = [consts.tile([P, HDL], bf, name=f"wkt{kt}") for kt in range(NKT)]
            wv_t = [consts.tile([P, HDL], bf, name=f"wvt{kt}") for kt in range(NKT)]
            for kt in range(NKT):
                nc.sync.dma_start(out=xt_t[kt][:, :], in_=xT_v[:, kt, :])
            for kt in range(NKT):
                nc.sync.dma_start(
                    out=wq_t[kt][:, 0:P], in_=wq_v[:, kt, 0:P]
                )
            for kt in range(NKT):
                nc.sync.dma_start(
                    out=wk_t[kt][:, 0:P], in_=wk_v[:, kt, 0:P]
                )
            for kt in range(NKT):
                nc.sync.dma_start(
                    out=wq_t[kt][:, P:HDL], in_=wq_v[:, kt, P:HDL]
                )
                nc.sync.dma_start(
                    out=wk_t[kt][:, P:HDL], in_=wk_v[:, kt, P:HDL]
                )
            mask_sb = consts.tile([P, 2, P], bf)
            nc.sync.dma_start(
                out=mask_sb[:, :, :],
                in_=mask_d[:, :].rearrange("p (a c) -> p a c", a=2),
            )
            for kt in range(NKT):
                nc.sync.dma_start(out=wv_t[kt][:, :], in_=wv_v[:, kt, :])
            wo_sb = consts.tile([P, NPT, D], bf)

            QT_sb = bigpool.tile([P, NPT, S], bf)
            KT_sb = bigpool.tile([P, NPT, S], bf)
            V_sb = bigpool.tile([P, NST, HL, 66], bf)
            OT_t = [
                [
                    bigpool.tile([P, QC], bf, name=f"ot{p}_{j}")
                    for j in range(NQC)
                ]
                for p in range(NPT)
            ]

            def proj_qk(ptile):
                for w_sb, dst in ((wq_t, QT_sb), (wk_t, KT_sb)):
                    for sc in range(NQC):
                        ps = ps_sc.tile([P, QC], f32, tag="ps_sc",
                                        name=f"pj{ptile}_{sc}")
                        for kt in range(NKT):
                            nc.tensor.matmul(
                                ps[:, :],
                                lhsT=w_sb[kt][:, ptile * P : (ptile + 1) * P],
                                rhs=xt_t[kt][:, sc * QC : (sc + 1) * QC],
                                start=(kt == 0),
                                stop=(kt == NKT - 1),
                            )
                        nc.vector.tensor_copy(
                            dst[:, ptile, sc * QC : (sc + 1) * QC], ps[:, :]
                        )

            def attention(pair, post_j=None, j_order=None):
                for j in (j_order if j_order is not None else range(NQC)):
                    nkt = 4 * j + 4
                    av = [
                        ps_av.tile([P, QC], f32, tag="ps_av",
                                   name=f"av{j}_{pair}_{h01}")
                        for h01 in range(2)
                    ]
                    for kt in range(nkt):
                        a = kt - 4 * j  # >=0: diagonal block alignment
                        off = P * a if a >= 0 else 0
                        # both heads' scores^T in one 2-bank psum tile
                        scp = ps_sc.tile([P, 2 * QC], f32, tag="ps_sc",
                                         name=f"sc{j}_{pair}_{kt}")
                        for h01 in range(2):
                            base = 64 * h01
                            nc.tensor.matmul(
                                scp[:, h01 * QC + off : (h01 + 1) * QC],
                                lhsT=KT_sb[
                                    base : base + 64, pair, kt * P : (kt + 1) * P
                                ],
                                rhs=QT_sb[
                                    base : base + 64, pair,
                                    j * QC + off : (j + 1) * QC,
                                ],
                                start=True,
                                stop=True,
                            )
                        pb = ppool.tile([P, 2 * QC], bf, tag="probs")
                        if off:
                            nc.scalar.activation(
                                out=pb[:, :].rearrange("p (h q) -> p h q", h=2)[
                                    :, :, off:QC
                                ],
                                in_=scp[:, :].rearrange("p (h q) -> p h q", h=2)[
                                    :, :, off:QC
                                ],
                                func=mybir.ActivationFunctionType.Exp,
                                scale=SCALE,
                            )
                        else:
                            nc.scalar.activation(
                                out=pb[:, :],
                                in_=scp[:, :],
                                func=mybir.ActivationFunctionType.Exp,
                                scale=SCALE,
                            )
                        if a >= 0:
                            nc.vector.tensor_mul(
                                pb[:, :].rearrange("p (h q) -> p h q", h=2)[
                                    :, :, off : off + P
                                ],
                                pb[:, :].rearrange("p (h q) -> p h q", h=2)[
                                    :, :, off : off + P
                                ],
                                mask_sb[:, :, :],
                            )
                        if debug and j == 0 and pair == 0 and kt == 0:
                            nc.sync.dma_start(
                                out=dbg_pb_d[:, :], in_=pb[:, QC : 2 * QC]
                            )
                        for h01 in range(2):
                            h = 2 * pair + h01
                            nc.tensor.matmul(
                                av[h01][0:65, off:QC],
                                lhsT=V_sb[:, kt, h, 0:65],
                                rhs=pb[:, h01 * QC + off : (h01 + 1) * QC],
                                start=(kt == 0),
                                stop=(kt == nkt - 1),
                            )
                    # epilogue: normalize by the ones-row sums.  Custom DVE
                    # ops only work at partition base 0, so copy the two
                    # sums rows into rows 0/32 of a base-0 tile first.
                    recin = recp.tile([33, QC], f32, tag="recin",
                                      name=f"ri{j}_{pair}")
                    nc.vector.memset(recin[:, :], 1.0)
                    for h01 in range(2):
                        nc.vector.tensor_copy(
                            recin[32 * h01 : 32 * h01 + 1, :], av[h01][64:65, :]
                        )
                    recfull = recp.tile([33, QC], f32, tag="recfull",
                                        name=f"rf{j}_{pair}")
                    nc.vector.reciprocal_approx_fast(
                        out=recfull[0:33, :], in_=recin[0:33, :]
                    )
                    # DRAM-bounce partition broadcast of the two recip rows
                    rd = dramp.tile([2, QC], f32, tag="rec_dram",
                                    name=f"rd{j}_{pair}")
                    nc.sync.dma_start(out=rd[0:1, :], in_=recfull[0:1, :])
                    nc.sync.dma_start(out=rd[1:2, :], in_=recfull[32:33, :])
                    bcs = spool.tile([P, QC], f32, tag="bcs")
                    for h01 in range(2):
                        bsrc = bass.AP(
                            tensor=rd.tensor,
                            offset=rd[h01 : h01 + 1, :].offset,
                            ap=[[0, 64], [1, QC]],
                        )
                        nc.sync.dma_start(
                            out=bcs[64 * h01 : 64 * h01 + 64, :], in_=bsrc
                        )
                    if debug and j == 0 and pair == 0:
                        nc.sync.dma_start(out=dbg_rec_d[0:33, :], in_=recfull[:, :])
                        dbg_av_sb = spool.tile([P, QC], f32, tag="dbg_av_sb")
                        nc.vector.tensor_copy(dbg_av_sb[0:65, :], av[1][0:65, :])
                        nc.sync.dma_start(
                            out=dbg_av_d[0:65, :], in_=dbg_av_sb[0:65, :]
                        )
                    for h01 in range(2):
                        base = 64 * h01
                        nc.vector.tensor_mul(
                            OT_t[pair][j][base : base + 64, :],
                            av[h01][0:64, :],
                            bcs[base : base + 64, :],
                        )
                    if post_j is not None:
                        post_j(j)

            def proj_v(st):
                ps = ps_sc.tile([P, QC], f32, tag="ps_sc", name=f"vp{st}")
                for kt in range(NKT):
                    nc.tensor.matmul(
                        ps[:, :],
                        lhsT=xt_t[kt][:, st * P : (st + 1) * P],
                        rhs=wv_t[kt][:, :],
                        start=(kt == 0),
                        stop=(kt == NKT - 1),
                    )
                nc.vector.tensor_copy(
                    V_sb[:, st, :, 0:64],
                    ps[:, :].rearrange("p (h d) -> p h d", h=HL),
                )

            # ---- phases 1+2 interleaved: attention(p) then PE filler work ----
            nc.vector.memset(V_sb[:, :, :, 64:65], 1.0)
            proj_qk(0)
            for st in range(NST):
                proj_v(st)
            def outproj_group(g):
                for st in range(4 * g, 4 * g + 4):
                    for nch in range(2):
                        ps = ps_sc.tile([P, QC], f32, tag="ps_sc",
                                        name=f"op{st}_{nch}")
                        for p in range(NPT):
                            nc.tensor.matmul(
                                ps[:, :],
                                lhsT=OT_t[p][st // 4][
                                    :, (st % 4) * P : (st % 4 + 1) * P
                                ],
                                rhs=wo_sb[:, p, nch * QC : (nch + 1) * QC],
                                start=(p == 0),
                                stop=(p == NPT - 1),
                            )
                        osb = opool.tile([P, QC], f32, tag="osb")
                        nc.vector.tensor_copy(osb[:, :], ps[:, :])
                        nc.sync.dma_start(
                            out=out_v[:, st, nch * QC : (nch + 1) * QC],
                            in_=osb[:, :],
                        )

            for pair in range(NPT):
                if pair + 1 == NPT:
                    nc.sync.dma_start(out=wo_sb[:, :, :], in_=wo_v[:, :, :])
                    # last pair runs j descending and each outproj group is
                    # delayed one j-iteration, so the final group (g=0)
                    # waits only on the smallest chunk's epilogue
                    attention(
                        pair,
                        post_j=lambda j: outproj_group(j + 1) if j + 1 < NQC
                        else None,
                        j_order=[3, 2, 1, 0],
                    )
                    outproj_group(0)
                else:
                    attention(pair)
                    proj_qk(pair + 1)

            if debug:
                nc.sync.dma_start(out=dbg_qt_d[:, :, :], in_=QT_sb[:, :, :])
                nc.sync.dma_start(out=dbg_kt_d[:, :, :], in_=KT_sb[:, :, :])
                nc.sync.dma_start(out=dbg_v_d[:, :, :, :], in_=V_sb[:, :, :, :])
                for p in range(NPT):
                    for j in range(NQC):
                        nc.sync.dma_start(
                            out=dbg_ot_d[:, p, j * QC : (j + 1) * QC],
                            in_=OT_t[p][j][:, :],
                        )


    nc.compile()
    return nc


def get_nc(debug=False):
    key = ("nc", debug)
    if key not in _CACHED:
        _CACHED[key] = _build_nc(debug)
    return _CACHED[key]


def make_core_inputs(x, W_q, W_k, W_v, W_o):
    """Per-core input dicts (numpy, bf16 where applicable)."""
    tri = np.triu(np.ones((P, P), np.float32))  # c>=r -> 1
    mask_np = np.concatenate([tri, tri], axis=1).astype(BF16)  # (P, 2P)
    in_maps = []
    for c in range(8):
        b, g = c // 2, c % 2
        hs = slice(g * HL, (g + 1) * HL)
        in_maps.append(
            {
                "xT": np.ascontiguousarray(x[b].T).astype(BF16),
                "wq": np.ascontiguousarray(
                    W_q[hs].transpose(1, 0, 2).reshape(D, HDL)
                ).astype(BF16),
                "wk": np.ascontiguousarray(
                    W_k[hs].transpose(1, 0, 2).reshape(D, HDL)
                ).astype(BF16),
                "wv": np.ascontiguousarray(
                    W_v[hs].transpose(1, 0, 2).reshape(D, HDL)
                ).astype(BF16),
                "wo": np.ascontiguousarray(W_o[hs].reshape(HDL, D)).astype(BF16),
                "mask": mask_np,
            }
        )
    return in_maps


def kernel(x, mask, W_q, W_k, W_v, W_o):
    from concourse.bass_utils import run_bass_kernel_spmd

    x = np.asarray(x, np.float32)
    nc = get_nc()
    in_maps = make_core_inputs(
        x, np.asarray(W_q), np.asarray(W_k), np.asarray(W_v), np.asarray(W_o)
    )
    res = run_bass_kernel_spmd(nc, in_maps, core_ids=list(range(8)))
    out = np.zeros((B, S, D), np.float32)
    for c in range(8):
        out[c // 2] += res.results[c]["out"]
    return out

